# revision 11
# baseline (speedup 1.0000x reference)
"""Trainium2 Bass kernel for nn_ContextualModule (contextual attention).

Sharding: 8 cores = 4 samples x 2 spatial halves (rows y in [0,32) / [32,64)).
Each core computes attention for a 34-row window y in [yr0-1, yr0+33) (one
halo row each side; out-of-grid rows are neutralized by data, see below).

Algorithm restructuring (validated bit-level vs reference in check_math.py):
  - Only rows with mm=1 (3x3 mask patch fully background, ~7.5% of 4096) get
    nonzero attention.  Host gathers the kernel matrix to [576, La] columns
    (La ~ 330 -> Lp=384 padded).  13x less matmul/softmax work.
  - The 3x3 score propagation is linear over the spatial axis -> folded into
    the FG operand on host (box-filtered shifted patches), so one matmul
    yields the propagated score directly.
  - score^T layout [p=spatial partitions, l_active=free]: softmax and argmax
    are native free-axis reductions.  Masked rows' softmax contribution is
    restored in closed form: S += (4096-La)*exp(-10*bias).
  - Two spare contraction rows fold data-dependent adds into the matmul:
    row 639 (ones) adds mmneg (pad columns -> -1e6), row 638 marks
    out-of-grid window rows (whole row -> -1e6 -> att row == 0).
  - att columns normalized in place (ACT copy, per-partition 1/S), PE
    transposes to [l, p], conv_transpose = 9 shifted matmuls against
    host-gathered flipped kernels Bg9.
  - argmax with reference tie semantics: monotone encodes off(l)-2^24 /
    offy(l)-2^24, selected by (score==rowmax) and min-reduced.
  - Matmuls in float32r (FP22 inputs, fp32 PSUM accumulation) at full rate.
"""

import numpy as np

B, C, H, W = 4, 64, 64, 64
L = H * W
EPS = 1.1920929e-07
TIMES = float(H * W) / float((H + 2) * (W + 2))
NEG = -1.0e6
BIGF = float(2 ** 24)
CS = 640            # padded contraction: 576 real + row 638 (invalid) + 639 (ones)
NPT = 17            # score p-tiles per core (34 window rows x 64)
NOT = 16            # own/output p-tiles per core (32 rows x 64)

_CACHE = {}


# ----------------------------------------------------------------------------
# host-side preparation
# ----------------------------------------------------------------------------

def _box3x3_zero(a):
    p = np.pad(a, [(0, 0)] * (a.ndim - 2) + [(1, 1), (1, 1)])
    t = p[..., 0:H, :] + p[..., 1:H + 1, :] + p[..., 2:H + 2, :]
    return t[..., 0:W] + t[..., 1:W + 1] + t[..., 2:W + 2]


def _split22(x):
    """Split into FP22-exact hi + lo so hi+lo == x to ~26 mantissa bits."""
    x = np.ascontiguousarray(x, np.float32)
    hi = (x.view(np.uint32) & np.uint32(0xFFFFFC00)).view(np.float32)
    lo = x - hi
    lo = (lo.view(np.uint32) & np.uint32(0xFFFFFC00)).view(np.float32)
    return hi, lo


def _host_prep(fg, bg, mk):
    fg = np.ascontiguousarray(fg, np.float32)
    bg = np.ascontiguousarray(bg, np.float32)
    mk = np.ascontiguousarray(mk, np.float32)
    m = mk[:, 0]
    bgm = bg * (1.0 - m[:, None])
    bgm_pad = np.pad(bgm, ((0, 0), (0, 0), (1, 1), (1, 1)))   # [B,C,66,66]
    m_pad = np.pad(m, ((0, 0), (1, 1), (1, 1)))
    msum = np.zeros((B, H, W), np.float32)
    for sy in range(3):
        for sx in range(3):
            msum += m_pad[:, sy:sy + H, sx:sx + W]
    mmb = (msum.reshape(B, L) == 0.0)
    actives = [np.nonzero(mmb[b])[0].astype(np.int64) for b in range(B)]
    Lmax = max(1, max(len(a) for a in actives))
    Lp = ((Lmax + 127) // 128) * 128
    nLc = Lp // 128

    # FPS: propagation-smoothed shifted FG patches  [B, 576, 4096] (s-major)
    fg_pad = np.pad(fg, ((0, 0), (0, 0), (1, 1), (1, 1)))
    FP = np.empty((B, 9, C, H, W), np.float32)
    for s in range(9):
        sy, sx = s // 3, s % 3
        FP[:, s] = fg_pad[:, :, sy:sy + H, sx:sx + W]
    FPS = _box3x3_zero(FP).reshape(B, 9 * C, L)

    ident = np.eye(128, dtype=np.float32)
    per_core = []
    for b in range(B):
        act = actives[b]
        La = len(act)
        yj, xj = act // W, act % W

        bp = bgm_pad[b]                      # [C,66,66]; [:, yj, xj] -> [C, La]
        kg = np.zeros((CS, Lp), np.float32)
        for s in range(9):
            sy, sx = s // 3, s % 3
            kg[s * C:(s + 1) * C, :La] = bp[:, yj + sy, xj + sx] + EPS
        kg[CS - 2, :] = NEG                  # invalid-window-row marker
        kg[CS - 1, La:] = NEG                # pad-column mask (mmneg)

        bg9 = np.zeros((Lp, 9 * C), np.float32)
        for d9 in range(9):
            dy, dx = d9 // 3, d9 % 3
            bg9[:La, d9 * C:(d9 + 1) * C] = \
                (bp[:, yj + 2 - dy, xj + 2 - dx] + EPS).T
        bg9 = np.ascontiguousarray(bg9.reshape(nLc, 128, 9 * C))

        # offset encodings, exact f32 replication of the reference int cast
        lf = act.astype(np.float32)
        v = (lf + np.float32(1.0)) * np.float32(TIMES) - np.float32(1.0)
        off = v.astype(np.int32)             # trunc toward zero
        offy = off // H
        rows = np.full((2, Lp), -BIGF, np.float32)   # pad -> l*=0 -> off 0
        rows[0, :La] = off.astype(np.float32) - BIGF
        rows[1, :La] = offy.astype(np.float32) - BIGF

        nmask = np.full((1, 1), float(L - La), np.float32)

        for half in range(2):
            yr0 = 32 * half
            ya = yr0 - 1                     # window rows y' = ya + w, w in [0,34)

            fpsx = np.zeros((CS, NPT * 128), np.float32)
            wlo, whi = (1, 34) if half == 0 else (0, 33)   # valid w range
            fpsx[:9 * C, wlo * 64:whi * 64] = \
                FPS[b][:, (ya + wlo) * 64:(ya + whi) * 64]
            fpsx[CS - 1, wlo * 64:whi * 64] = 1.0          # ones row (valid cols)
            winv = 0 if half == 0 else 33                  # the out-of-grid row
            fpsx[CS - 2, winv * 64:(winv + 1) * 64] = 1.0  # invalid marker

            g0 = yr0 * W
            pg = np.arange(NOT * 128)
            mo = m[b].reshape(L)[g0:g0 + NOT * 128]
            cols = np.empty((2, 128, NOT), np.float32)
            cols[0] = (mo / 9.0).reshape(NOT, 128).T
            cols[1] = (1.0 - mo).reshape(NOT, 128).T

            pw = np.arange(NPT * 128)
            pos = np.empty((2, 128, NPT), np.float32)
            pos[0] = (ya + pw // W).astype(np.float32).reshape(NPT, 128).T
            pos[1] = (pw % W).astype(np.float32).reshape(NPT, 128).T

            fgt = np.ascontiguousarray(
                fg[b].reshape(C, L)[:, g0:g0 + NOT * 128].T.reshape(NOT, 128, C))

            per_core.append({
                "fps": np.ascontiguousarray(fpsx.reshape(5, 128, NPT * 128)),
                "kg": np.ascontiguousarray(kg.reshape(5, 128, Lp)),
                "bg9": bg9,
                "fgt": fgt,
                "rows": rows,
                "cols": cols,
                "pos": pos,
                "nmask": nmask,
                "ident": ident,
            })
    return per_core, Lp


# ----------------------------------------------------------------------------
# device program (uniform across all 8 cores)
# ----------------------------------------------------------------------------

def _build(Lp):
    import concourse.bass as bass
    import concourse.bacc as bacc
    import concourse.tile as tile
    from concourse import mybir

    f32 = mybir.dt.float32
    f32r = mybir.dt.float32r
    nLc = Lp // 128
    Alu = mybir.AluOpType
    Act = mybir.ActivationFunctionType
    X = mybir.AxisListType.X

    nc = bacc.Bacc("TRN2", target_bir_lowering=False, debug=False,
                   enable_asserts=False)

    fps_d = nc.dram_tensor("fps", [5, 128, NPT * 128], f32, kind="ExternalInput")
    kg_d = nc.dram_tensor("kg", [5, 128, Lp], f32, kind="ExternalInput")
    bg9_d = nc.dram_tensor("bg9", [nLc, 128, 9 * C], f32r, kind="ExternalInput")
    fgt_d = nc.dram_tensor("fgt", [NOT, 128, C], f32, kind="ExternalInput")
    rows_d = nc.dram_tensor("rows", [2, Lp], f32, kind="ExternalInput")
    cols_d = nc.dram_tensor("cols", [2, 128, NOT], f32, kind="ExternalInput")
    pos_d = nc.dram_tensor("pos", [2, 128, NPT], f32, kind="ExternalInput")
    nmask_d = nc.dram_tensor("nmask", [1, 1], f32, kind="ExternalInput")
    ident_d = nc.dram_tensor("ident", [128, 128], f32, kind="ExternalInput")
    atto_d = nc.dram_tensor("atto", [NOT, 128, C], f32, kind="ExternalOutput")
    offo_d = nc.dram_tensor("offo", [128, NPT, 2], f32, kind="ExternalOutput")

    def bcast(ap, p=128):
        return bass.AP(tensor=ap.tensor, offset=ap.offset,
                       ap=[[0, p], list(ap.ap[-1])])

    with tile.TileContext(nc) as tc:
        with (
            tc.tile_pool(name="big", bufs=1) as big,
            tc.tile_pool(name="work", bufs=4) as work,
            tc.tile_pool(name="cand", bufs=3) as candp,
            tc.tile_pool(name="stat", bufs=12) as stat,
            tc.tile_pool(name="acc", bufs=1) as accp,
            tc.tile_pool(name="psS", bufs=3, space="PSUM") as psS,
            tc.tile_pool(name="psT", bufs=2, space="PSUM") as psT,
            tc.tile_pool(name="psR", bufs=2, space="PSUM") as psR,
            tc.tile_pool(name="psF", bufs=1, space="PSUM") as psF,
        ):
            # ---------------- input DMAs ----------------
            kg_sb = big.tile([128, 5, Lp], f32)
            nc.sync.dma_start(out=kg_sb, in_=kg_d.ap().rearrange("k p f -> p k f"))
            fps_pieces = []
            PIECES = [(0, 4), (4, 8), (8, 12), (12, NPT)]
            for (t0, t1) in PIECES:
                fp = big.tile([128, 5, (t1 - t0) * 128], f32, tag=f"fps{t0}")
                nc.sync.dma_start(
                    out=fp,
                    in_=fps_d.ap()[:, :, t0 * 128:t1 * 128]
                        .rearrange("k p f -> p k f"))
                fps_pieces.append((t0, t1, fp))
            bg9_sb = big.tile([128, nLc, 9, C], f32r)
            nc.sync.dma_start(out=bg9_sb,
                              in_=bg9_d.ap().rearrange("k p (d c) -> p k d c", c=C))
            fgt_sb = big.tile([128, NOT, C], f32)
            nc.sync.dma_start(out=fgt_sb, in_=fgt_d.ap().rearrange("t p c -> p t c"))
            identr_sb = big.tile([128, 128], f32r)
            nc.sync.dma_start(out=identr_sb, in_=ident_d.ap().bitcast(f32r))
            identf_sb = big.tile([128, 128], f32)
            nc.sync.dma_start(out=identf_sb, in_=ident_d.ap())
            cols_sb = big.tile([128, 2, NOT], f32)
            nc.sync.dma_start(out=cols_sb, in_=cols_d.ap().rearrange("k p t -> p k t"))
            pos_sb = big.tile([128, 2, NPT], f32)
            nc.sync.dma_start(out=pos_sb, in_=pos_d.ap().rearrange("k p t -> p k t"))
            offmB = big.tile([128, Lp], f32)
            nc.gpsimd.dma_start(out=offmB, in_=bcast(rows_d.ap()[0]))
            offymB = big.tile([128, Lp], f32)
            nc.gpsimd.dma_start(out=offymB, in_=bcast(rows_d.ap()[1]))
            nmaskB = big.tile([128, 1], f32)
            nc.gpsimd.dma_start(out=nmaskB, in_=bcast(nmask_d.ap()[0]))

            esb = accp.tile([128, NPT, Lp], f32r)
            attx = accp.tile([128, nLc, 34, 66], f32r)
            nc.gpsimd.memset(attx[:].bitcast(f32), 0.0)
            omin = accp.tile([128, NPT], f32)
            ymin = accp.tile([128, NPT], f32)
            rec_sb = accp.tile([64, NOT * 128], f32)
            out_sb = accp.tile([128, NOT, C], f32)
            off_sb = accp.tile([128, NPT, 2], f32)

            def fps_slice(i, k):
                for (t0, t1, fp) in fps_pieces:
                    if t0 <= i < t1:
                        return fp[:, k, (i - t0) * 128:(i - t0 + 1) * 128]
                raise AssertionError

            # ---------------- phase A: score + softmax + argmax ----------------
            for i in range(NPT):
                ps = psS.tile([128, Lp], f32)
                for k in range(5):
                    # native fp32 matmul (4-pass): ~2e-7 rel, no argmax flips
                    nc.tensor.matmul(ps,
                                     fps_slice(i, k),
                                     kg_sb[:, k, :],
                                     start=(k == 0), stop=(k == 4))
                rmax = stat.tile([128, 1], f32, tag="rmax")
                nc.vector.reduce_max(rmax, ps, axis=X)
                bias = stat.tile([128, 1], f32, tag="bias")
                nc.vector.tensor_scalar_max(bias, rmax, 0.0)
                negb = stat.tile([128, 1], f32, tag="negb")
                nc.vector.tensor_scalar_mul(negb, bias, -10.0)
                s0 = stat.tile([128, 1], f32, tag="s0")
                nc.scalar.activation(esb[:, i, :], ps, Act.Exp,
                                     bias=negb, scale=10.0, accum_out=s0)
                corr = stat.tile([128, 1], f32, tag="corr")
                nc.scalar.activation(corr, negb, Act.Exp)
                ssum = stat.tile([128, 1], f32, tag="ssum")
                nc.vector.scalar_tensor_tensor(ssum, corr, nmaskB[:, 0:1], s0,
                                               op0=Alu.mult, op1=Alu.add)
                nc.vector.tensor_scalar_max(ssum, ssum, 1e-30)
                invs = stat.tile([128, 1], f32, tag="invs")
                nc.vector.reciprocal(invs, ssum)
                nc.scalar.mul(esb[:, i, :], esb[:, i, :], invs)   # att normalize
                ca = candp.tile([128, Lp], f32, tag="ca")
                nc.vector.scalar_tensor_tensor(ca, ps, rmax, offmB,
                                               op0=Alu.is_equal, op1=Alu.mult)
                nc.vector.tensor_reduce(omin[:, i:i + 1], ca, axis=X, op=Alu.min)
                cb = candp.tile([128, Lp], f32, tag="cb")
                nc.vector.scalar_tensor_tensor(cb, ps, rmax, offymB,
                                               op0=Alu.is_equal, op1=Alu.mult)
                nc.vector.tensor_reduce(ymin[:, i:i + 1], cb, axis=X, op=Alu.min)

            # ---------------- phase B: transpose att to [l, p] ----------------
            for i in range(NPT):
                for k in range(nLc):
                    pt = psT.tile([128, 128], f32r)
                    nc.tensor.transpose(pt, esb[:, i, k * 128:(k + 1) * 128],
                                        identr_sb)
                    nc.vector.tensor_copy(
                        attx[:, k, 2 * i:2 * i + 2, 1:65], pt)

            # ---------------- phase C: conv_transpose ----------------
            nmm = 9 * nLc
            for j in range(4):
                pr = psR.tile([64, 512], f32)
                n = 0
                for d9 in range(9):
                    dy, dx = d9 // 3, d9 % 3
                    for k in range(nLc):
                        rhs = attx[:, k, 8 * j + dy:8 * j + dy + 8, dx:dx + 64]
                        nc.tensor.matmul(pr,
                                         bg9_sb[:, k, d9, :], rhs,
                                         start=(n == 0), stop=(n == nmm - 1))
                        n += 1
                nc.vector.tensor_copy(rec_sb[:, j * 512:(j + 1) * 512], pr)

            # ---------------- phase D: final combine ----------------
            for t in range(NOT):
                pf = psF.tile([128, C], f32)
                nc.tensor.transpose(pf, rec_sb[:, t * 128:(t + 1) * 128],
                                    identf_sb[:64, :64])
                tmp = work.tile([128, C], f32, tag="tmp")
                nc.vector.tensor_scalar_mul(tmp, fgt_sb[:, t, :],
                                            cols_sb[:, 1, t:t + 1])
                nc.vector.scalar_tensor_tensor(out_sb[:, t, :], pf,
                                               cols_sb[:, 0, t:t + 1], tmp,
                                               op0=Alu.mult, op1=Alu.add)

            # ---------------- phase E: offsets ----------------
            offT = work.tile([128, NPT], f32, tag="offT")
            nc.vector.tensor_scalar_add(offT, omin, BIGF)
            offyT = work.tile([128, NPT], f32, tag="offyT")
            nc.vector.tensor_scalar_add(offyT, ymin, BIGF)
            offxT = work.tile([128, NPT], f32, tag="offxT")
            nc.vector.scalar_tensor_tensor(offxT, offyT, -64.0, offT,
                                           op0=Alu.mult, op1=Alu.add)
            nc.vector.tensor_sub(off_sb[:, :, 0], offyT, pos_sb[:, 0, :])
            nc.vector.tensor_sub(off_sb[:, :, 1], offxT, pos_sb[:, 1, :])

            nc.sync.dma_start(out=atto_d.ap().rearrange("t p c -> p t c"),
                              in_=out_sb)
            nc.sync.dma_start(out=offo_d.ap(), in_=off_sb)

    nc.compile()
    return nc


# ----------------------------------------------------------------------------
# execution
# ----------------------------------------------------------------------------

def _run(inputs, trace=False):
    fg = np.asarray(inputs["foreground"], np.float32)
    bg = np.asarray(inputs["background"], np.float32)
    mk = np.asarray(inputs["mask"], np.float32)

    per_core, Lp = _host_prep(fg, bg, mk)
    if Lp > 512:
        return _numpy_fallback(fg, bg, mk), None

    if Lp not in _CACHE:
        _CACHE[Lp] = _build(Lp)
    nc = _CACHE[Lp]

    from concourse.bass_utils import run_bass_kernel_spmd
    res = run_bass_kernel_spmd(nc, per_core, core_ids=list(range(8)),
                               trace=trace)

    attended = np.empty((B, C, H, W), np.float32)
    offout = np.empty((B, 2, H, W), np.float32)
    for core in range(8):
        b, half = core // 2, core % 2
        yr0 = 32 * half
        r = res.results[core]
        att = r["atto"].reshape(NOT * 128, C)          # [p, c], p = yrel*64+x
        attended[b, :, yr0:yr0 + 32, :] = att.T.reshape(C, 32, W)
        offo = r["offo"]                               # [128, NPT, 2]
        o = offo.transpose(1, 0, 2).reshape(NPT * 128, 2)  # [w*64+x, 2]
        offout[b, :, yr0:yr0 + 32, :] = \
            o[64:64 + 32 * 64].T.reshape(2, 32, W)     # keep w in [1,33)
    return (attended, offout), res


def _numpy_fallback(fg, bg, mk):
    """Exact reference reimplementation (host); only for degenerate masks."""
    SH = [(dy, dx) for dy in range(3) for dx in range(3)]
    att_out = np.empty((B, C, H, W), np.float32)
    off_out = np.empty((B, 2, H, W), np.float32)
    for b in range(B):
        fgb, bgb, m = fg[b], bg[b] * (1.0 - mk[b]), mk[b]
        bg_pad = np.pad(bgb, ((0, 0), (1, 1), (1, 1)))
        m_pad = np.pad(m, ((0, 0), (1, 1), (1, 1)))
        fg_pad = np.pad(fgb, ((0, 0), (1, 1), (1, 1)))
        Kst = np.stack([bg_pad[:, dy:dy + H, dx:dx + W] for dy, dx in SH], -1)
        K = Kst.transpose(1, 2, 0, 3).reshape(L, C, 3, 3) + EPS
        msum = sum(m_pad[0, dy:dy + H, dx:dx + W] for dy, dx in SH)
        mm = (msum.reshape(L) == 0.0).astype(np.float32)
        FP = np.stack([fg_pad[:, dy:dy + H, dx:dx + W] for dy, dx in SH], 1)
        score = (K.reshape(L, C * 9) @ FP.reshape(C * 9, L)).reshape(L, H, W)
        sp = np.pad(score, ((0, 0), (1, 1), (1, 1)))
        score = sum(sp[:, dy:dy + H, dx:dx + W] for dy, dx in SH)
        score = score * mm[:, None, None]
        z = score * np.float32(10.0)
        z = z - z.max(0, keepdims=True)
        e = np.exp(z)
        att = e / e.sum(0, keepdims=True) * mm[:, None, None]
        offset = np.argmax(att.reshape(L, L), 0).reshape(H, W)
        off = ((offset + 1).astype(np.float32) * np.float32(TIMES)
               - np.float32(1.0)).astype(np.int32)
        off2 = np.stack([off // H, off % W], 0).astype(np.float32)
        ap = np.pad(att, ((0, 0), (1, 1), (1, 1)))
        rec = np.zeros((C, H, W), np.float32)
        for dy, dx in SH:
            A = ap[:, dy:dy + H, dx:dx + W].reshape(L, L)
            rec += (K[:, :, 2 - dy, 2 - dx].T @ A).reshape(C, H, W)
        rec = rec * m / 9.0
        att_out[b] = rec * m + fgb * (1.0 - m)
        yy, xx = np.meshgrid(np.arange(H), np.arange(W), indexing="ij")
        off_out[b] = off2 - np.stack([yy, xx], 0)
    return att_out, off_out


def kernel(**inputs):
    out, _ = _run(inputs, trace=False)
    return out


# revision 12
# speedup vs baseline: 1.0193x; 1.0193x over previous
"""Trainium2 Bass kernel for nn_ContextualModule (contextual attention).

Sharding: 8 cores = 4 samples x 2 spatial halves (rows y in [0,32) / [32,64)).
Each core computes attention for a 34-row window y in [yr0-1, yr0+33) (one
halo row each side; out-of-grid rows are neutralized by data, see below).

Algorithm restructuring (validated bit-level vs reference in check_math.py):
  - Only rows with mm=1 (3x3 mask patch fully background, ~7.5% of 4096) get
    nonzero attention.  Host gathers the kernel matrix to [576, La] columns
    (La ~ 330 -> Lp=384 padded).  13x less matmul/softmax work.
  - The 3x3 score propagation is linear over the spatial axis -> folded into
    the FG operand on host (box-filtered shifted patches), so one matmul
    yields the propagated score directly.
  - score^T layout [p=spatial partitions, l_active=free]: softmax and argmax
    are native free-axis reductions.  Masked rows' softmax contribution is
    restored in closed form: S += (4096-La)*exp(-10*bias).
  - Two spare contraction rows fold data-dependent adds into the matmul:
    row 639 (ones) adds mmneg (pad columns -> -1e6), row 638 marks
    out-of-grid window rows (whole row -> -1e6 -> att row == 0).
  - att columns normalized in place (ACT copy, per-partition 1/S), PE
    transposes to [l, p], conv_transpose = 9 shifted matmuls against
    host-gathered flipped kernels Bg9.
  - argmax with reference tie semantics: monotone encodes off(l)-2^24 /
    offy(l)-2^24, selected by (score==rowmax) and min-reduced.
  - Matmuls in float32r (FP22 inputs, fp32 PSUM accumulation) at full rate.
"""

import numpy as np

B, C, H, W = 4, 64, 64, 64
L = H * W
EPS = 1.1920929e-07
TIMES = float(H * W) / float((H + 2) * (W + 2))
NEG = -1.0e6
BIGF = float(2 ** 24)
CS = 640            # padded contraction: 576 real + row 638 (invalid) + 639 (ones)
NPT = 17            # score p-tiles per core (34 window rows x 64)
NOT = 16            # own/output p-tiles per core (32 rows x 64)

_CACHE = {}


# ----------------------------------------------------------------------------
# host-side preparation
# ----------------------------------------------------------------------------

def _box3x3_zero(a):
    p = np.pad(a, [(0, 0)] * (a.ndim - 2) + [(1, 1), (1, 1)])
    t = p[..., 0:H, :] + p[..., 1:H + 1, :] + p[..., 2:H + 2, :]
    return t[..., 0:W] + t[..., 1:W + 1] + t[..., 2:W + 2]


def _split22(x):
    """Split into FP22-exact hi + lo so hi+lo == x to ~26 mantissa bits."""
    x = np.ascontiguousarray(x, np.float32)
    hi = (x.view(np.uint32) & np.uint32(0xFFFFFC00)).view(np.float32)
    lo = x - hi
    lo = (lo.view(np.uint32) & np.uint32(0xFFFFFC00)).view(np.float32)
    return hi, lo


def _host_prep(fg, bg, mk):
    fg = np.ascontiguousarray(fg, np.float32)
    bg = np.ascontiguousarray(bg, np.float32)
    mk = np.ascontiguousarray(mk, np.float32)
    m = mk[:, 0]
    bgm = bg * (1.0 - m[:, None])
    bgm_pad = np.pad(bgm, ((0, 0), (0, 0), (1, 1), (1, 1)))   # [B,C,66,66]
    m_pad = np.pad(m, ((0, 0), (1, 1), (1, 1)))
    msum = np.zeros((B, H, W), np.float32)
    for sy in range(3):
        for sx in range(3):
            msum += m_pad[:, sy:sy + H, sx:sx + W]
    mmb = (msum.reshape(B, L) == 0.0)
    actives = [np.nonzero(mmb[b])[0].astype(np.int64) for b in range(B)]
    Lmax = max(1, max(len(a) for a in actives))
    Lp = ((Lmax + 127) // 128) * 128
    nLc = Lp // 128

    # FPS: propagation-smoothed shifted FG patches  [B, 576, 4096] (s-major)
    fg_pad = np.pad(fg, ((0, 0), (0, 0), (1, 1), (1, 1)))
    FP = np.empty((B, 9, C, H, W), np.float32)
    for s in range(9):
        sy, sx = s // 3, s % 3
        FP[:, s] = fg_pad[:, :, sy:sy + H, sx:sx + W]
    FPS = _box3x3_zero(FP).reshape(B, 9 * C, L)

    ident = np.eye(128, dtype=np.float32)
    per_core = []
    for b in range(B):
        act = actives[b]
        La = len(act)
        yj, xj = act // W, act % W

        bp = bgm_pad[b]                      # [C,66,66]; [:, yj, xj] -> [C, La]
        kg = np.zeros((CS, Lp), np.float32)
        for s in range(9):
            sy, sx = s // 3, s % 3
            kg[s * C:(s + 1) * C, :La] = bp[:, yj + sy, xj + sx] + EPS
        kg[CS - 2, :] = NEG                  # invalid-window-row marker
        kg[CS - 1, La:] = NEG                # pad-column mask (mmneg)

        bg9 = np.zeros((Lp, 9 * C), np.float32)
        for d9 in range(9):
            dy, dx = d9 // 3, d9 % 3
            bg9[:La, d9 * C:(d9 + 1) * C] = \
                (bp[:, yj + 2 - dy, xj + 2 - dx] + EPS).T
        bg9 = np.ascontiguousarray(bg9.reshape(nLc, 128, 9 * C))

        # offset encodings, exact f32 replication of the reference int cast
        lf = act.astype(np.float32)
        v = (lf + np.float32(1.0)) * np.float32(TIMES) - np.float32(1.0)
        off = v.astype(np.int32)             # trunc toward zero
        offy = off // H
        rows = np.full((2, Lp), -BIGF, np.float32)   # pad -> l*=0 -> off 0
        rows[0, :La] = off.astype(np.float32) - BIGF
        rows[1, :La] = offy.astype(np.float32) - BIGF

        nmask = np.full((1, 1), float(L - La), np.float32)

        for half in range(2):
            yr0 = 32 * half
            ya = yr0 - 1                     # window rows y' = ya + w, w in [0,34)

            fpsx = np.zeros((CS, NPT * 128), np.float32)
            wlo, whi = (1, 34) if half == 0 else (0, 33)   # valid w range
            fpsx[:9 * C, wlo * 64:whi * 64] = \
                FPS[b][:, (ya + wlo) * 64:(ya + whi) * 64]
            fpsx[CS - 1, wlo * 64:whi * 64] = 1.0          # ones row (valid cols)
            winv = 0 if half == 0 else 33                  # the out-of-grid row
            fpsx[CS - 2, winv * 64:(winv + 1) * 64] = 1.0  # invalid marker

            g0 = yr0 * W
            pg = np.arange(NOT * 128)
            mo = m[b].reshape(L)[g0:g0 + NOT * 128]
            cols = np.empty((2, 128, NOT), np.float32)
            cols[0] = (mo / 9.0).reshape(NOT, 128).T
            cols[1] = (1.0 - mo).reshape(NOT, 128).T

            pw = np.arange(NPT * 128)
            pos = np.empty((2, 128, NPT), np.float32)
            pos[0] = (ya + pw // W).astype(np.float32).reshape(NPT, 128).T
            pos[1] = (pw % W).astype(np.float32).reshape(NPT, 128).T

            fgt = np.ascontiguousarray(
                fg[b].reshape(C, L)[:, g0:g0 + NOT * 128].T.reshape(NOT, 128, C))

            per_core.append({
                "fps": np.ascontiguousarray(fpsx.reshape(5, 128, NPT * 128)),
                "kg": np.ascontiguousarray(kg.reshape(5, 128, Lp)),
                "bg9": bg9,
                "fgt": fgt,
                "rows": rows,
                "cols": cols,
                "pos": pos,
                "nmask": nmask,
                "ident": ident,
            })
    return per_core, Lp


# ----------------------------------------------------------------------------
# device program (uniform across all 8 cores)
# ----------------------------------------------------------------------------

def _build(Lp):
    import concourse.bass as bass
    import concourse.bacc as bacc
    import concourse.tile as tile
    from concourse import mybir

    f32 = mybir.dt.float32
    f32r = mybir.dt.float32r
    nLc = Lp // 128
    Alu = mybir.AluOpType
    Act = mybir.ActivationFunctionType
    X = mybir.AxisListType.X

    nc = bacc.Bacc("TRN2", target_bir_lowering=False, debug=False,
                   enable_asserts=False)

    fps_d = nc.dram_tensor("fps", [5, 128, NPT * 128], f32, kind="ExternalInput")
    kg_d = nc.dram_tensor("kg", [5, 128, Lp], f32, kind="ExternalInput")
    bg9_d = nc.dram_tensor("bg9", [nLc, 128, 9 * C], f32r, kind="ExternalInput")
    fgt_d = nc.dram_tensor("fgt", [NOT, 128, C], f32, kind="ExternalInput")
    rows_d = nc.dram_tensor("rows", [2, Lp], f32, kind="ExternalInput")
    cols_d = nc.dram_tensor("cols", [2, 128, NOT], f32, kind="ExternalInput")
    pos_d = nc.dram_tensor("pos", [2, 128, NPT], f32, kind="ExternalInput")
    nmask_d = nc.dram_tensor("nmask", [1, 1], f32, kind="ExternalInput")
    ident_d = nc.dram_tensor("ident", [128, 128], f32, kind="ExternalInput")
    atto_d = nc.dram_tensor("atto", [NOT, 128, C], f32, kind="ExternalOutput")
    offo_d = nc.dram_tensor("offo", [128, NPT, 2], f32, kind="ExternalOutput")

    def bcast(ap, p=128):
        return bass.AP(tensor=ap.tensor, offset=ap.offset,
                       ap=[[0, p], list(ap.ap[-1])])

    with tile.TileContext(nc) as tc:
        with (
            tc.tile_pool(name="big", bufs=1) as big,
            tc.tile_pool(name="work", bufs=4) as work,
            tc.tile_pool(name="cand", bufs=3) as candp,
            tc.tile_pool(name="stat", bufs=12) as stat,
            tc.tile_pool(name="acc", bufs=1) as accp,
            tc.tile_pool(name="psS", bufs=3, space="PSUM") as psS,
            tc.tile_pool(name="psT", bufs=2, space="PSUM") as psT,
            tc.tile_pool(name="psR", bufs=2, space="PSUM") as psR,
            tc.tile_pool(name="psF", bufs=1, space="PSUM") as psF,
        ):
            # ---------------- input DMAs ----------------
            kg_sb = big.tile([128, 5, Lp], f32)
            for k in range(5):
                nc.sync.dma_start(out=kg_sb[:, k, :], in_=kg_d.ap()[k])
            fps_pieces = []
            PIECES = [(0, 2), (2, 4), (4, 8), (8, 12), (12, NPT)]
            for (t0, t1) in PIECES:
                fp = big.tile([128, 5, (t1 - t0) * 128], f32, tag=f"fps{t0}")
                for k in range(5):
                    nc.sync.dma_start(
                        out=fp[:, k, :],
                        in_=fps_d.ap()[k][:, t0 * 128:t1 * 128])
                fps_pieces.append((t0, t1, fp))
            bg9_sb = big.tile([128, nLc, 9, C], f32r)
            nc.sync.dma_start(out=bg9_sb,
                              in_=bg9_d.ap().rearrange("k p (d c) -> p k d c", c=C))
            fgt_sb = big.tile([128, NOT, C], f32)
            nc.sync.dma_start(out=fgt_sb, in_=fgt_d.ap().rearrange("t p c -> p t c"))
            identr_sb = big.tile([128, 128], f32r)
            nc.sync.dma_start(out=identr_sb, in_=ident_d.ap().bitcast(f32r))
            identf_sb = big.tile([128, 128], f32)
            nc.sync.dma_start(out=identf_sb, in_=ident_d.ap())
            cols_sb = big.tile([128, 2, NOT], f32)
            nc.sync.dma_start(out=cols_sb, in_=cols_d.ap().rearrange("k p t -> p k t"))
            pos_sb = big.tile([128, 2, NPT], f32)
            nc.sync.dma_start(out=pos_sb, in_=pos_d.ap().rearrange("k p t -> p k t"))
            offmB = big.tile([128, Lp], f32)
            nc.gpsimd.dma_start(out=offmB, in_=bcast(rows_d.ap()[0]))
            offymB = big.tile([128, Lp], f32)
            nc.gpsimd.dma_start(out=offymB, in_=bcast(rows_d.ap()[1]))
            nmaskB = big.tile([128, 1], f32)
            nc.gpsimd.dma_start(out=nmaskB, in_=bcast(nmask_d.ap()[0]))

            esb = accp.tile([128, NPT, Lp], f32r)
            attx = accp.tile([128, nLc, 34, 66], f32r)
            # only the x-border columns (0 and 65) are never written by the
            # E-transpose evicts; zero just those instead of the whole buffer
            nc.gpsimd.memset(attx[:, :, :, 0:1].bitcast(f32), 0.0)
            nc.gpsimd.memset(attx[:, :, :, 65:66].bitcast(f32), 0.0)
            omin = accp.tile([128, NPT], f32)
            ymin = accp.tile([128, NPT], f32)
            rec_sb = accp.tile([64, NOT * 128], f32)
            out_sb = accp.tile([128, NOT, C], f32)
            off_sb = accp.tile([128, NPT, 2], f32)

            def fps_slice(i, k):
                for (t0, t1, fp) in fps_pieces:
                    if t0 <= i < t1:
                        return fp[:, k, (i - t0) * 128:(i - t0 + 1) * 128]
                raise AssertionError

            # ---------------- phase A: score + softmax + argmax ----------------
            for i in range(NPT):
                ps = psS.tile([128, Lp], f32)
                for k in range(5):
                    # native fp32 matmul (4-pass): ~2e-7 rel, no argmax flips
                    nc.tensor.matmul(ps,
                                     fps_slice(i, k),
                                     kg_sb[:, k, :],
                                     start=(k == 0), stop=(k == 4))
                rmax = stat.tile([128, 1], f32, tag="rmax")
                nc.vector.reduce_max(rmax, ps, axis=X)
                bias = stat.tile([128, 1], f32, tag="bias")
                nc.vector.tensor_scalar_max(bias, rmax, 0.0)
                negb = stat.tile([128, 1], f32, tag="negb")
                nc.vector.tensor_scalar_mul(negb, bias, -10.0)
                s0 = stat.tile([128, 1], f32, tag="s0")
                nc.scalar.activation(esb[:, i, :], ps, Act.Exp,
                                     bias=negb, scale=10.0, accum_out=s0)
                corr = stat.tile([128, 1], f32, tag="corr")
                nc.scalar.activation(corr, negb, Act.Exp)
                ssum = stat.tile([128, 1], f32, tag="ssum")
                nc.vector.scalar_tensor_tensor(ssum, corr, nmaskB[:, 0:1], s0,
                                               op0=Alu.mult, op1=Alu.add)
                nc.vector.tensor_scalar_max(ssum, ssum, 1e-30)
                invs = stat.tile([128, 1], f32, tag="invs")
                nc.vector.reciprocal(invs, ssum)
                nc.scalar.mul(esb[:, i, :], esb[:, i, :], invs)   # att normalize
                ca = candp.tile([128, Lp], f32, tag="ca")
                nc.vector.scalar_tensor_tensor(ca, ps, rmax, offmB,
                                               op0=Alu.is_equal, op1=Alu.mult)
                nc.vector.tensor_reduce(omin[:, i:i + 1], ca, axis=X, op=Alu.min)
                cb = candp.tile([128, Lp], f32, tag="cb")
                nc.vector.scalar_tensor_tensor(cb, ps, rmax, offymB,
                                               op0=Alu.is_equal, op1=Alu.mult)
                nc.vector.tensor_reduce(ymin[:, i:i + 1], cb, axis=X, op=Alu.min)

            # ---------------- phase B: transpose att to [l, p] ----------------
            for i in range(NPT):
                for k in range(nLc):
                    pt = psT.tile([128, 128], f32r)
                    nc.tensor.transpose(pt, esb[:, i, k * 128:(k + 1) * 128],
                                        identr_sb)
                    nc.vector.tensor_copy(
                        attx[:, k, 2 * i:2 * i + 2, 1:65], pt)

            # ---------------- phase C: conv_transpose ----------------
            nmm = 9 * nLc
            for j in range(4):
                pr = psR.tile([64, 512], f32)
                n = 0
                for d9 in range(9):
                    dy, dx = d9 // 3, d9 % 3
                    for k in range(nLc):
                        rhs = attx[:, k, 8 * j + dy:8 * j + dy + 8, dx:dx + 64]
                        nc.tensor.matmul(pr,
                                         bg9_sb[:, k, d9, :], rhs,
                                         start=(n == 0), stop=(n == nmm - 1))
                        n += 1
                nc.vector.tensor_copy(rec_sb[:, j * 512:(j + 1) * 512], pr)

            # ---------------- phase D: final combine ----------------
            for t in range(NOT):
                pf = psF.tile([128, C], f32)
                nc.tensor.transpose(pf, rec_sb[:, t * 128:(t + 1) * 128],
                                    identf_sb[:64, :64])
                tmp = work.tile([128, C], f32, tag="tmp")
                nc.vector.tensor_scalar_mul(tmp, fgt_sb[:, t, :],
                                            cols_sb[:, 1, t:t + 1])
                nc.vector.scalar_tensor_tensor(out_sb[:, t, :], pf,
                                               cols_sb[:, 0, t:t + 1], tmp,
                                               op0=Alu.mult, op1=Alu.add)

            # ---------------- phase E: offsets ----------------
            offT = work.tile([128, NPT], f32, tag="offT")
            nc.vector.tensor_scalar_add(offT, omin, BIGF)
            offyT = work.tile([128, NPT], f32, tag="offyT")
            nc.vector.tensor_scalar_add(offyT, ymin, BIGF)
            offxT = work.tile([128, NPT], f32, tag="offxT")
            nc.vector.scalar_tensor_tensor(offxT, offyT, -64.0, offT,
                                           op0=Alu.mult, op1=Alu.add)
            nc.vector.tensor_sub(off_sb[:, :, 0], offyT, pos_sb[:, 0, :])
            nc.vector.tensor_sub(off_sb[:, :, 1], offxT, pos_sb[:, 1, :])

            nc.sync.dma_start(out=atto_d.ap().rearrange("t p c -> p t c"),
                              in_=out_sb)
            nc.sync.dma_start(out=offo_d.ap(), in_=off_sb)

    nc.compile()
    return nc


# ----------------------------------------------------------------------------
# execution
# ----------------------------------------------------------------------------

def _run(inputs, trace=False):
    fg = np.asarray(inputs["foreground"], np.float32)
    bg = np.asarray(inputs["background"], np.float32)
    mk = np.asarray(inputs["mask"], np.float32)

    per_core, Lp = _host_prep(fg, bg, mk)
    if Lp > 512:
        return _numpy_fallback(fg, bg, mk), None

    if Lp not in _CACHE:
        _CACHE[Lp] = _build(Lp)
    nc = _CACHE[Lp]

    from concourse.bass_utils import run_bass_kernel_spmd
    res = run_bass_kernel_spmd(nc, per_core, core_ids=list(range(8)),
                               trace=trace)

    attended = np.empty((B, C, H, W), np.float32)
    offout = np.empty((B, 2, H, W), np.float32)
    for core in range(8):
        b, half = core // 2, core % 2
        yr0 = 32 * half
        r = res.results[core]
        att = r["atto"].reshape(NOT * 128, C)          # [p, c], p = yrel*64+x
        attended[b, :, yr0:yr0 + 32, :] = att.T.reshape(C, 32, W)
        offo = r["offo"]                               # [128, NPT, 2]
        o = offo.transpose(1, 0, 2).reshape(NPT * 128, 2)  # [w*64+x, 2]
        offout[b, :, yr0:yr0 + 32, :] = \
            o[64:64 + 32 * 64].T.reshape(2, 32, W)     # keep w in [1,33)
    return (attended, offout), res


def _numpy_fallback(fg, bg, mk):
    """Exact reference reimplementation (host); only for degenerate masks."""
    SH = [(dy, dx) for dy in range(3) for dx in range(3)]
    att_out = np.empty((B, C, H, W), np.float32)
    off_out = np.empty((B, 2, H, W), np.float32)
    for b in range(B):
        fgb, bgb, m = fg[b], bg[b] * (1.0 - mk[b]), mk[b]
        bg_pad = np.pad(bgb, ((0, 0), (1, 1), (1, 1)))
        m_pad = np.pad(m, ((0, 0), (1, 1), (1, 1)))
        fg_pad = np.pad(fgb, ((0, 0), (1, 1), (1, 1)))
        Kst = np.stack([bg_pad[:, dy:dy + H, dx:dx + W] for dy, dx in SH], -1)
        K = Kst.transpose(1, 2, 0, 3).reshape(L, C, 3, 3) + EPS
        msum = sum(m_pad[0, dy:dy + H, dx:dx + W] for dy, dx in SH)
        mm = (msum.reshape(L) == 0.0).astype(np.float32)
        FP = np.stack([fg_pad[:, dy:dy + H, dx:dx + W] for dy, dx in SH], 1)
        score = (K.reshape(L, C * 9) @ FP.reshape(C * 9, L)).reshape(L, H, W)
        sp = np.pad(score, ((0, 0), (1, 1), (1, 1)))
        score = sum(sp[:, dy:dy + H, dx:dx + W] for dy, dx in SH)
        score = score * mm[:, None, None]
        z = score * np.float32(10.0)
        z = z - z.max(0, keepdims=True)
        e = np.exp(z)
        att = e / e.sum(0, keepdims=True) * mm[:, None, None]
        offset = np.argmax(att.reshape(L, L), 0).reshape(H, W)
        off = ((offset + 1).astype(np.float32) * np.float32(TIMES)
               - np.float32(1.0)).astype(np.int32)
        off2 = np.stack([off // H, off % W], 0).astype(np.float32)
        ap = np.pad(att, ((0, 0), (1, 1), (1, 1)))
        rec = np.zeros((C, H, W), np.float32)
        for dy, dx in SH:
            A = ap[:, dy:dy + H, dx:dx + W].reshape(L, L)
            rec += (K[:, :, 2 - dy, 2 - dx].T @ A).reshape(C, H, W)
        rec = rec * m / 9.0
        att_out[b] = rec * m + fgb * (1.0 - m)
        yy, xx = np.meshgrid(np.arange(H), np.arange(W), indexing="ij")
        off_out[b] = off2 - np.stack([yy, xx], 0)
    return att_out, off_out


def kernel(**inputs):
    out, _ = _run(inputs, trace=False)
    return out
